# revision 1
# baseline (speedup 1.0000x reference)
"""Trainium2 Bass kernel for the bidirectional LSTM sampled-softmax loss.

Math (B=16, L=512, D=256, N = B*L = 8192 rows):
  f        = feats * mask           (positions >= seq_len zeroed)
  G_dir    = h_dir @ f_flat.T       (N x N GEMM, dir in {fw, bw})
  den_dir  = exp(G_dir).sum(-1)
  num_dir  = exp(rowsum(h_dir * tgt_dir))   (tgt = f shifted +-1)
  seq_b    = sum_j mask * num/den ; loss = mean_b(-log(seq_b)/len_b)

Sharding: 1024 query rows per core = exactly 2 whole sequences per core
(b = 2m, 2m+1), f_flat.T replicated -> row sums and per-sequence sums are
core-local, no collectives. Each core returns 4 scalars
(-log(seq)/(16*len) for [fw seq0, fw seq1, bw seq0, bw seq1]); the host
just adds them up.

Device kernel per core:
  - GEMM in bf16 (K=256 as 2 accumulating matmuls, N=512 per matmul) into
    PSUM tiles of (128, 2048) = 4 banks, double-buffered.
  - exp via ScalarE in-place on PSUM with accum_out folding the row-sum
    into the activation instruction (no separate reduce over the 8192-wide
    exp rows).
  - numerator dots via one fused DVE tensor_tensor_reduce per row block.
  - cross-partition sums via two tiny PE matmuls (ones / group-selector).
  - log + (-1/(16*len)) scaling on device.
"""

import sys

for _p in ("/opt/trn_rl_repo", "/root/.axon_site/_ro/trn_rl_repo"):
    if _p not in sys.path:
        sys.path.append(_p)

import numpy as np
import ml_dtypes

BF16 = ml_dtypes.bfloat16

B, L, D = 16, 512, 256
N = B * L           # 8192 total rows/keys
M = 8               # cores
ROWS = N // M       # 1024 query rows per core (per direction)
NRB = 16            # row blocks of 128 per core: 8 fw + 8 bw
NCG = 4             # key column groups
CG = N // NCG       # 2048 keys per group
NT = CG // 512      # 512-wide matmul tiles per group

_NC_CACHE = {}


def _build_nc():
    import concourse.bass as bass
    import concourse.mybir as mybir
    from concourse import bacc
    from concourse.tile import TileContext

    fp32 = mybir.dt.float32
    bf16 = mybir.dt.bfloat16
    Alu = mybir.AluOpType
    Act = mybir.ActivationFunctionType

    nc = bacc.Bacc("TRN2", target_bir_lowering=False)

    d_flatT = nc.dram_tensor("flatT", [D, N], bf16, kind="ExternalInput")
    d_hT = nc.dram_tensor("hT", [D, 2 * ROWS], bf16, kind="ExternalInput")
    d_hrow = nc.dram_tensor("hrow", [128, NRB * D], bf16, kind="ExternalInput")
    d_tgt = nc.dram_tensor("tgt", [128, NRB * D], bf16, kind="ExternalInput")
    d_mask = nc.dram_tensor("maskv", [128, NRB], fp32, kind="ExternalInput")
    d_sel = nc.dram_tensor("sel", [NRB, 4], fp32, kind="ExternalInput")
    d_il = nc.dram_tensor("invlen", [4, 1], fp32, kind="ExternalInput")
    d_ones = nc.dram_tensor("ones", [128, 1], fp32, kind="ExternalInput")
    d_out = nc.dram_tensor("out", [4, 1], fp32, kind="ExternalOutput")

    with TileContext(nc) as tc:
        with tc.tile_pool(name="const", bufs=1) as cp, \
             tc.tile_pool(name="flat", bufs=2) as fpool, \
             tc.tile_pool(name="work", bufs=2) as wp, \
             tc.tile_pool(name="ps", bufs=2, space="PSUM") as pp:

            hT_sb = cp.tile([128, 2, 2 * ROWS], bf16, tag="hT")
            nc.sync.dma_start(
                out=hT_sb[:],
                in_=d_hT[:, :].rearrange("(k p) c -> p k c", p=128),
            )
            hrow_sb = cp.tile([128, NRB * D], bf16, tag="hrow")
            nc.sync.dma_start(out=hrow_sb[:], in_=d_hrow[:, :])
            tgt_sb = cp.tile([128, NRB * D], bf16, tag="tgt")
            nc.sync.dma_start(out=tgt_sb[:], in_=d_tgt[:, :])
            mask_sb = cp.tile([128, NRB], fp32, tag="mask")
            nc.sync.dma_start(out=mask_sb[:], in_=d_mask[:, :])
            sel_sb = cp.tile([NRB, 4], fp32, tag="sel")
            nc.sync.dma_start(out=sel_sb[:], in_=d_sel[:, :])
            il_sb = cp.tile([4, 1], fp32, tag="il")
            nc.sync.dma_start(out=il_sb[:], in_=d_il[:, :])
            ones_sb = cp.tile([128, 1], fp32, tag="ones")
            nc.sync.dma_start(out=ones_sb[:], in_=d_ones[:, :])

            den_parts = cp.tile([128, NRB * NCG], fp32, tag="denp")
            numdot = cp.tile([128, NRB], fp32, tag="numdot")

            # Numerator row-dots: numdot[p, rb] = h[row] . tgt[row].
            # Independent of the GEMM loop; DVE runs these under it.
            for rb in range(NRB):
                scr = wp.tile([128, D], fp32, tag="scr")
                nc.vector.tensor_mul(
                    out=scr[:],
                    in0=hrow_sb[:, rb * D:(rb + 1) * D],
                    in1=tgt_sb[:, rb * D:(rb + 1) * D],
                )
                nc.vector.reduce_sum(
                    numdot[:, rb:rb + 1],
                    scr[:],
                    axis=mybir.AxisListType.X,
                )

            # Main loop: G = h @ flatT per (key-group, row-block), exp+rowsum.
            for cg in range(NCG):
                flat_sb = fpool.tile([128, 2, CG], bf16, tag="flat")
                nc.sync.dma_start(
                    out=flat_sb[:],
                    in_=d_flatT[:, cg * CG:(cg + 1) * CG].rearrange(
                        "(k p) c -> p k c", p=128
                    ),
                )
                for rb in range(NRB):
                    pt = pp.tile([128, CG], fp32, tag="g")
                    for ct in range(NT):
                        for k in range(2):
                            nc.tensor.matmul(
                                pt[:, ct * 512:(ct + 1) * 512],
                                hT_sb[:, k, rb * 128:(rb + 1) * 128],
                                flat_sb[:, k, ct * 512:(ct + 1) * 512],
                                start=(k == 0),
                                stop=(k == 1),
                            )
                    col = rb * NCG + cg
                    nc.scalar.activation(
                        pt[:],
                        pt[:],
                        Act.Exp,
                        accum_out=den_parts[:, col:col + 1],
                    )

            # Final reduction stage (tiny).
            den_all = cp.tile([128, NRB], fp32, tag="den")
            nc.vector.reduce_sum(
                den_all[:, :, None],
                den_parts[:].rearrange("p (r g) -> p r g", g=NCG),
                axis=mybir.AxisListType.X,
            )
            num_all = cp.tile([128, NRB], fp32, tag="num")
            nc.scalar.activation(num_all[:], numdot[:], Act.Exp)
            recip = cp.tile([128, NRB], fp32, tag="recip")
            nc.vector.reciprocal(recip[:], den_all[:])
            ratio = cp.tile([128, NRB], fp32, tag="ratio")
            nc.vector.tensor_mul(out=ratio[:], in0=num_all[:], in1=recip[:])
            nc.vector.tensor_mul(out=ratio[:], in0=ratio[:], in1=mask_sb[:])

            # blocksums[rb] = sum_p ratio[p, rb]  (K=128, M=16, N=1)
            bs_ps = pp.tile([NRB, 1], fp32, tag="g")
            nc.tensor.matmul(bs_ps[:], ratio[:], ones_sb[:], start=True, stop=True)
            bs_sb = cp.tile([NRB, 1], fp32, tag="bs")
            nc.scalar.copy(bs_sb[:], bs_ps[:])

            # seq sums: sel.T @ blocksums  (K=16, M=4, N=1)
            ss_ps = pp.tile([4, 1], fp32, tag="g")
            nc.tensor.matmul(ss_ps[:], sel_sb[:], bs_sb[:], start=True, stop=True)

            logv = cp.tile([4, 1], fp32, tag="logv")
            nc.scalar.activation(logv[:], ss_ps[:], Act.Ln)
            loss = cp.tile([4, 1], fp32, tag="loss")
            nc.vector.tensor_mul(out=loss[:], in0=logv[:], in1=il_sb[:])
            nc.sync.dma_start(out=d_out[:, :], in_=loss[:])

    nc.compile()
    return nc


def _get_nc():
    if "nc" not in _NC_CACHE:
        _NC_CACHE["nc"] = _build_nc()
    return _NC_CACHE["nc"]


def _prep_in_maps(feats, hidden, seq_lens):
    feats = np.asarray(feats, np.float32)
    hidden = np.asarray(hidden, np.float32)
    seq_lens = np.asarray(seq_lens).astype(np.int64).reshape(B)

    mask = np.arange(L)[None, :] < seq_lens[:, None]            # (B, L)
    f = feats * mask[..., None].astype(np.float32)              # (B, L, D)
    h_fw = np.ascontiguousarray(hidden[..., :D]).reshape(N, D)
    h_bw = np.ascontiguousarray(hidden[..., D:]).reshape(N, D)
    zero = np.zeros((B, 1, D), np.float32)
    fw_tgt = np.concatenate([f[:, 1:], zero], axis=1).reshape(N, D)
    bw_tgt = np.concatenate([zero, f[:, :-1]], axis=1).reshape(N, D)
    flat = f.reshape(N, D)

    flatT_bf = np.ascontiguousarray(flat.T).astype(BF16)        # (256, 8192)
    mask_flat = mask.reshape(N).astype(np.float32)
    lens = seq_lens.astype(np.float64)

    sel = np.zeros((NRB, 4), np.float32)
    for k in range(NRB):
        sel[k, k // 4] = 1.0
    ones = np.ones((128, 1), np.float32)

    in_maps = []
    for m in range(M):
        rs = slice(m * ROWS, (m + 1) * ROWS)
        hT = np.concatenate([h_fw[rs].T, h_bw[rs].T], axis=1).astype(BF16)
        hcat = np.concatenate([h_fw[rs], h_bw[rs]], axis=0)      # (2048, 256)
        tcat = np.concatenate([fw_tgt[rs], bw_tgt[rs]], axis=0)
        hrow = np.ascontiguousarray(
            hcat.reshape(NRB, 128, D).transpose(1, 0, 2).reshape(128, NRB * D)
        ).astype(BF16)
        tgtr = np.ascontiguousarray(
            tcat.reshape(NRB, 128, D).transpose(1, 0, 2).reshape(128, NRB * D)
        ).astype(BF16)
        mv = np.ascontiguousarray(mask_flat[rs].reshape(8, 128).T)  # (128, 8)
        maskv = np.concatenate([mv, mv], axis=1).astype(np.float32)
        l0, l1 = lens[2 * m], lens[2 * m + 1]
        invlen = np.array(
            [[-1.0 / (16 * l0)], [-1.0 / (16 * l1)],
             [-1.0 / (16 * l0)], [-1.0 / (16 * l1)]], np.float32
        )
        in_maps.append(dict(
            flatT=flatT_bf,
            hT=np.ascontiguousarray(hT),
            hrow=hrow,
            tgt=tgtr,
            maskv=maskv,
            sel=sel,
            invlen=invlen,
            ones=ones,
        ))
    return in_maps


def _run(in_maps, trace=False):
    from concourse.bass_utils import run_bass_kernel_spmd

    nc = _get_nc()
    return run_bass_kernel_spmd(nc, in_maps, list(range(M)), trace=trace)


def kernel(feats, hidden, seq_lens):
    in_maps = _prep_in_maps(feats, hidden, seq_lens)
    res = _run(in_maps).results
    fw = 0.0
    bw = 0.0
    for m in range(M):
        o = np.asarray(res[m]["out"], np.float32).reshape(4)
        fw += float(o[0]) + float(o[1])
        bw += float(o[2]) + float(o[3])
    return (np.asarray(fw, np.float32), np.asarray(bw, np.float32))



# revision 2
# speedup vs baseline: 1.1954x; 1.1954x over previous
"""Trainium2 Bass kernel for the bidirectional LSTM sampled-softmax loss, v2.

Math (B=16, L=512, D=256, N = B*L = 8192 rows):
  f        = feats * mask           (positions >= seq_len zeroed)
  G_dir    = h_dir @ f_flat.T       (N x N GEMM, dir in {fw, bw})
  den_dir  = exp(G_dir).sum(-1)
  num_dir  = exp(rowsum(h_dir * tgt_dir))   (tgt = f shifted +-1)
  seq_b    = sum_j mask * num/den ; loss = mean_b(-log(seq_b)/len_b)

The end-to-end cost of this problem under the axon tunnel is dominated by
host->device transfer (~12.5 ms/MB) and per-call dispatch, not by the
device kernel (~0.3 ms on silicon).  v2 therefore:

  - sends only the GEMM operands, in fp8 (e3m4, inputs scaled by 8 so the
    PE computes 64*G; ScalarE folds the 1/64 into exp's scale):
    per-core hT (256x2048) + per-core f-slice (256x1024) = 0.75 MB/core,
    6.3 MB total on the wire vs 58.8 MB for v1;
  - never replicates f from the host (host-side replication costs 8x on
    the wire): each core uploads its 1024-key slice and the kernel
    AllGathers the full 2 MB f matrix on-chip over NeuronLink;
  - computes the tiny O(N*D) numerator dots (0.1% of FLOPs) and the final
    ratio/log reduction on the host in float64;
  - caches the jitted PJRT callable across calls (the stock
    run_bass_kernel_spmd path re-traces and re-serializes the whole Bass
    module into the HLO on every call).

Device kernel per core (its 1024 rows x all 8192 keys, fw+bw):
  - AllGather the 8 f-slices into an internal Shared DRAM buffer.
  - G tile = hT.T @ flat in fp8e3 (K=256 as 2 accumulating matmuls,
    moving operand 1024 wide) into PSUM (128, 2048) fp32, 4 banks,
    double-buffered.
  - exp via ScalarE in-place on PSUM, scale=1/64, accum_out folding the
    row-sum into the activation; reduce the 4 column-group partials with
    one DVE reduce at the end; DMA the (128, 16) den block out.
"""

import sys

for _p in ("/opt/trn_rl_repo", "/root/.axon_site/_ro/trn_rl_repo"):
    if _p not in sys.path:
        sys.path.append(_p)

import numpy as np
import ml_dtypes

FP8 = ml_dtypes.float8_e3m4
SCALE = 8.0          # operands scaled by 8 -> PE computes 64*G; exp scale=1/64

B, L, D = 16, 512, 256
N = B * L            # 8192 total rows/keys
M = 8                # cores
ROWS = N // M        # 1024 query rows per core (per direction)
NRB = 16             # row blocks of 128 per core: 8 fw + 8 bw
NCG = 4              # key column groups of 2048
CG = N // NCG        # 2048 keys per group

_CACHE = {}


def _build_nc():
    import concourse.mybir as mybir
    from concourse import bacc
    from concourse.tile import TileContext

    fp32 = mybir.dt.float32
    fp8 = mybir.dt.float8e3
    Act = mybir.ActivationFunctionType

    nc = bacc.Bacc("TRN2", target_bir_lowering=False, num_devices=M)

    d_hT = nc.dram_tensor("hT", [D, 2 * ROWS], fp8, kind="ExternalInput")
    d_fsl = nc.dram_tensor("fsl", [D, ROWS], fp8, kind="ExternalInput")
    d_den = nc.dram_tensor("den", [128, NRB], fp32, kind="ExternalOutput")

    with TileContext(nc) as tc:
        with tc.tile_pool(name="dram", bufs=1, space="DRAM") as dram, \
             tc.tile_pool(name="const", bufs=1) as cp, \
             tc.tile_pool(name="flat", bufs=2) as fpool, \
             tc.tile_pool(name="ps", bufs=2, space="PSUM") as pp:

            # AllGather the f slices into the full (D, N) matrix, laid out
            # as 8 contiguous (D, 1024) blocks.
            in_bounce = dram.tile([D, ROWS], fp8)
            gath = dram.tile([M, D, ROWS], fp8, addr_space="Shared")
            nc.gpsimd.dma_start(in_bounce[:], d_fsl[:, :])
            nc.gpsimd.collective_compute(
                "AllGather",
                mybir.AluOpType.bypass,
                replica_groups=[list(range(M))],
                ins=[in_bounce[:].opt()],
                outs=[gath[:].opt()],
            )

            hT_sb = cp.tile([128, 2, 2 * ROWS], fp8, tag="hT")
            nc.sync.dma_start(
                out=hT_sb[:],
                in_=d_hT[:, :].rearrange("(k p) c -> p k c", p=128),
            )

            den_parts = cp.tile([128, NRB * NCG], fp32, tag="denp")

            for cg in range(NCG):
                flat_sb = fpool.tile([128, 2, CG], fp8, tag="flat")
                for half in range(2):
                    blk = 2 * cg + half
                    nc.sync.dma_start(
                        out=flat_sb[:, :, half * ROWS:(half + 1) * ROWS],
                        in_=gath[blk, :, :].rearrange("(k p) c -> p k c", p=128),
                    )
                for rb in range(NRB):
                    pt = pp.tile([128, CG], fp32, tag="g")
                    for ct in range(4):
                        for k in range(2):
                            nc.tensor.matmul(
                                pt[:, ct * 512:(ct + 1) * 512],
                                hT_sb[:, k, rb * 128:(rb + 1) * 128],
                                flat_sb[:, k, ct * 512:(ct + 1) * 512],
                                start=(k == 0),
                                stop=(k == 1),
                            )
                    col = rb * NCG + cg
                    nc.scalar.activation(
                        pt[:],
                        pt[:],
                        Act.Exp,
                        scale=1.0 / (SCALE * SCALE),
                        accum_out=den_parts[:, col:col + 1],
                    )

            den_all = cp.tile([128, NRB], fp32, tag="den")
            nc.vector.reduce_sum(
                den_all[:, :, None],
                den_parts[:].rearrange("p (r g) -> p r g", g=NCG),
                axis=mybir.AxisListType.X,
            )
            nc.sync.dma_start(out=d_den[:, :], in_=den_all[:])

    nc.compile()
    return nc


def _get_runner():
    """Build the Bass module and the jitted PJRT callable once, reuse on
    every call (the stock path re-traces and re-lowers per call)."""
    if "runner" in _CACHE:
        return _CACHE["runner"]

    import jax
    import concourse.mybir as mybir
    from concourse.bass2jax import (
        _bass_exec_p,
        install_neuronx_cc_hook,
        partition_id_tensor,
    )
    from jax.sharding import Mesh, PartitionSpec
    from jax.experimental.shard_map import shard_map

    nc = _build_nc()
    install_neuronx_cc_hook()
    assert nc.dbg_addr is None or not nc.dbg_callbacks

    partition_name = (
        nc.partition_id_tensor.name if nc.partition_id_tensor else None
    )
    in_names, out_names, out_avals = [], [], []
    for alloc in nc.m.functions[0].allocations:
        if not isinstance(alloc, mybir.MemoryLocationSet):
            continue
        name = alloc.memorylocations[0].name
        if alloc.kind == "ExternalInput":
            if name != partition_name and name != (
                nc.dbg_addr.name if nc.dbg_addr is not None else None
            ):
                in_names.append(name)
        elif alloc.kind == "ExternalOutput":
            shape = tuple(alloc.tensor_shape)
            dtype = mybir.dt.np(alloc.dtype)
            out_names.append(name)
            out_avals.append(jax.core.ShapedArray(shape, dtype))
    n_params = len(in_names)
    n_outs = len(out_avals)

    all_names = list(in_names) + list(out_names)
    extra_zero: list[np.ndarray] = []
    if nc.dbg_addr is not None:
        all_names.append(nc.dbg_addr.name)
        extra_zero.append(np.zeros((1, 2), np.uint32))
    if partition_name is not None:
        all_names.append(partition_name)

    def _body(*args):
        operands = list(args)
        for z in extra_zero:
            operands.append(z)
        if partition_name is not None:
            operands.append(partition_id_tensor())
        outs = _bass_exec_p.bind(
            *operands,
            out_avals=tuple(out_avals),
            in_names=tuple(all_names),
            out_names=tuple(out_names),
            lowering_input_output_aliases=(),
            sim_require_finite=True,
            sim_require_nnan=True,
            nc=nc,
        )
        return tuple(outs)

    devices = jax.devices()[:M]
    mesh = Mesh(np.asarray(devices), ("core",))
    donate = tuple(range(n_params, n_params + n_outs))
    sharded = jax.jit(
        shard_map(
            _body,
            mesh=mesh,
            in_specs=(PartitionSpec("core"),) * (n_params + n_outs),
            out_specs=(PartitionSpec("core"),) * n_outs,
            check_rep=False,
        ),
        donate_argnums=donate,
        keep_unused=True,
    )
    zero_out_shapes = [
        ((M * a.shape[0], *a.shape[1:]), a.dtype) for a in out_avals
    ]
    out_shapes = [tuple(a.shape) for a in out_avals]
    _CACHE["runner"] = (sharded, in_names, out_names, zero_out_shapes, out_shapes)
    return _CACHE["runner"]


def _prep_in_maps(feats, hidden, seq_lens):
    """Returns (in_maps for the device, host-side context for the final
    ratio/log stage)."""
    feats = np.asarray(feats, np.float32)
    hidden = np.asarray(hidden, np.float32)
    seq_lens = np.asarray(seq_lens).astype(np.int64).reshape(B)

    mask = np.arange(L)[None, :] < seq_lens[:, None]            # (B, L)
    f = feats * mask[..., None].astype(np.float32)              # (B, L, D)
    h_fw = hidden[..., :D]                                      # (B, L, D)
    h_bw = hidden[..., D:]

    # numerator dots on host (0.1% of the FLOPs), float32 in, float64 out
    ndot_fw = np.zeros((B, L), np.float64)
    ndot_bw = np.zeros((B, L), np.float64)
    ndot_fw[:, :-1] = np.einsum(
        "bjd,bjd->bj", h_fw[:, :-1], f[:, 1:], dtype=np.float64
    )
    ndot_bw[:, 1:] = np.einsum(
        "bjd,bjd->bj", h_bw[:, 1:], f[:, :-1], dtype=np.float64
    )

    flatT8 = np.ascontiguousarray(
        (f.reshape(N, D).T * SCALE)
    ).astype(FP8)                                               # (256, 8192)
    hf = h_fw.reshape(N, D)
    hb = h_bw.reshape(N, D)

    in_maps = []
    for m in range(M):
        rs = slice(m * ROWS, (m + 1) * ROWS)
        hT = np.concatenate([hf[rs].T, hb[rs].T], axis=1) * SCALE
        in_maps.append(dict(
            hT=np.ascontiguousarray(hT).astype(FP8),
            fsl=np.ascontiguousarray(flatT8[:, rs]),
        ))
    ctx = dict(
        num_fw=np.exp(ndot_fw),
        num_bw=np.exp(ndot_bw),
        mask=mask.astype(np.float64),
        lens=seq_lens.astype(np.float64),
    )
    return in_maps, ctx


def _run(in_maps):
    """One device round: upload fp8 operands, AllGather + GEMM + exp-rowsum
    on 8 cores, download the (8192, 2) denominator matrix."""
    import jax

    sharded, in_names, out_names, zero_out_shapes, out_shapes = _get_runner()
    concat_in = [
        np.concatenate([im[name] for im in in_maps], axis=0)
        for name in in_names
    ]
    zeros = [np.zeros(shape, dt) for shape, dt in zero_out_shapes]
    out = sharded(*concat_in, *zeros)
    res = []
    for m in range(M):
        res.append({
            name: np.asarray(out[i]).reshape(M, *out_shapes[i])[m]
            for i, name in enumerate(out_names)
        })
    return res


def _finish(res, ctx):
    den = np.stack([np.asarray(r["den"], np.float64) for r in res])  # (M,128,16)
    # den[m, p, rb]: rb<8 -> fw row m*1024 + rb*128 + p; rb>=8 -> bw likewise
    den_fw = den[:, :, :8].transpose(0, 2, 1).reshape(B, L)
    den_bw = den[:, :, 8:].transpose(0, 2, 1).reshape(B, L)

    mask = ctx["mask"]
    lens = ctx["lens"]
    fw_seq = (ctx["num_fw"] / den_fw * mask).sum(axis=1)
    bw_seq = (ctx["num_bw"] / den_bw * mask).sum(axis=1)
    fw_loss = np.mean(-np.log(fw_seq) / lens)
    bw_loss = np.mean(-np.log(bw_seq) / lens)
    return (np.asarray(fw_loss, np.float32), np.asarray(bw_loss, np.float32))


def kernel(feats, hidden, seq_lens):
    in_maps, ctx = _prep_in_maps(feats, hidden, seq_lens)
    res = _run(in_maps)
    return _finish(res, ctx)
